# revision 14
# baseline (speedup 1.0000x reference)
"""ViT classifier forward pass on 8 Trainium2 NeuronCores (data-parallel over batch).

Full inputs in, full outputs out. Internally: batch 32 -> 4 images per core.
Per-image token order is [196 patches, CLS, pad] (attention is permutation
equivariant; pos_emb is permuted to match, head reads CLS at its slot).
"""

import os
from contextlib import ExitStack

import numpy as np

import concourse.bass as bass
import concourse.tile as tile
import concourse.mybir as mybir
from concourse import bacc
from concourse.bass_utils import run_bass_kernel_spmd
from concourse.masks import make_identity

f32 = mybir.dt.float32
bf16 = mybir.dt.bfloat16
AF = mybir.ActivationFunctionType
ALU = mybir.AluOpType
AX = mybir.AxisListType

P = 128
N_CORES = 8
B = 4          # images per core
D = 768
H = 12
DK = 64
F = 3072
L = 12
NCLS = 1000
S = 197        # real tokens per image
EPS = 1e-5
SCALE = 0.125  # 1/sqrt(64)

NT = (126, 71)     # rows per (image, j) chunk: j=0 tokens 0..125, j=1 tokens 126..196
TOFF = (0, 126)    # token offset of chunk j within the image
CLSROW = 70        # CLS = row 70 of chunk j=1 (token index 196)
DT = D // P        # 6 d-tiles
FT = F // P        # 24 f-tiles
TPACK = B * S      # 788 packed tokens in transposed (free) layout
PATCH_ROWS = (126, 70)   # patch-matmul rows per chunk (no CLS)


def _bcast_row(nc, dst, src_ap, parts):
    """DMA-broadcast a [N]-shaped DRAM row to [parts, N] SBUF."""
    ap = bass.AP(tensor=src_ap.tensor, offset=src_ap.offset,
                 ap=[[0, parts]] + [list(a) for a in src_ap.ap])
    nc.sync.dma_start(out=dst, in_=ap)


def build():
    nc = bacc.Bacc("TRN2", target_bir_lowering=False, debug=False,
                   num_devices=N_CORES)

    img_d = nc.dram_tensor("img", [B, 3, 224, 224], f32, kind="ExternalInput")
    projw_d = nc.dram_tensor("proj_w", [D, D], f32, kind="ExternalInput")
    projb_d = nc.dram_tensor("proj_b", [D], f32, kind="ExternalInput")
    cls_d = nc.dram_tensor("cls_token", [1, 1, D], f32, kind="ExternalInput")
    pos_d = nc.dram_tensor("pos_emb", [1, S, D], f32, kind="ExternalInput")
    ln1s_d = nc.dram_tensor("ln1_s", [L, D], f32, kind="ExternalInput")
    ln1b_d = nc.dram_tensor("ln1_b", [L, D], f32, kind="ExternalInput")
    wq_d = nc.dram_tensor("wq", [L, D, D], f32, kind="ExternalInput")
    bq_d = nc.dram_tensor("bq", [L, D], f32, kind="ExternalInput")
    wk_d = nc.dram_tensor("wk", [L, D, D], f32, kind="ExternalInput")
    bk_d = nc.dram_tensor("bk", [L, D], f32, kind="ExternalInput")
    wv_d = nc.dram_tensor("wv", [L, D, D], f32, kind="ExternalInput")
    bv_d = nc.dram_tensor("bv", [L, D], f32, kind="ExternalInput")
    wo_d = nc.dram_tensor("wo", [L, D, D], f32, kind="ExternalInput")
    bo_d = nc.dram_tensor("bo", [L, D], f32, kind="ExternalInput")
    ln2s_d = nc.dram_tensor("ln2_s", [L, D], f32, kind="ExternalInput")
    ln2b_d = nc.dram_tensor("ln2_b", [L, D], f32, kind="ExternalInput")
    w1_d = nc.dram_tensor("w1", [L, D, F], f32, kind="ExternalInput")
    b1_d = nc.dram_tensor("b1", [L, F], f32, kind="ExternalInput")
    w2_d = nc.dram_tensor("w2", [L, F, D], f32, kind="ExternalInput")
    b2_d = nc.dram_tensor("b2", [L, D], f32, kind="ExternalInput")
    fns_d = nc.dram_tensor("fnorm_s", [D], f32, kind="ExternalInput")
    fnb_d = nc.dram_tensor("fnorm_b", [D], f32, kind="ExternalInput")
    hw_d = nc.dram_tensor("head_w", [D, NCLS], f32, kind="ExternalInput")
    hb_d = nc.dram_tensor("head_b", [NCLS], f32, kind="ExternalInput")
    out_d = nc.dram_tensor("out", [B, NCLS], f32, kind="ExternalOutput")

    with ExitStack() as ctx:
        tc = ctx.enter_context(tile.TileContext(nc))

        singles = ctx.enter_context(tc.tile_pool(name="singles", bufs=1))
        ident = singles.tile([P, P], bf16)
        make_identity(nc, ident[:])
        eps_t = singles.tile([P, 1], f32)
        nc.vector.memset(eps_t[:], EPS)
        x = singles.tile([P, B, 2, D], f32)          # residual stream

        stat = ctx.enter_context(tc.tile_pool(name="stat", bufs=4))

        pmm = ctx.enter_context(tc.tile_pool(name="pmm", bufs=3, space="PSUM"))
        ptr = ctx.enter_context(tc.tile_pool(name="ptr", bufs=2, space="PSUM"))
        psc = ctx.enter_context(tc.tile_pool(name="psc", bufs=2, space="PSUM"))
        pao = ctx.enter_context(tc.tile_pool(name="pao", bufs=1, space="PSUM"))

        def transpose_block(dst, src, rows):
            """dst[:, off:off+rows] (128 partitions) <- src[0:rows, 0:128].T"""
            t = ptr.tile([P, P], bf16, tag="tr")
            nc.tensor.transpose(t[0:P, 0:rows], src, ident[0:rows, 0:rows])
            nc.vector.tensor_copy(out=dst, in_=t[:, 0:rows])

        def layernorm_to_T(src_getter, s_bc, b_bc, xT, n_tok_off):
            """LN over feature dim for all 8 chunks; write bf16 transposed into
            xT[:, k, n_tok_off(i,j) : +rows]."""
            for i in range(B):
                for j in range(2):
                    rows = NT[j]
                    xs = src_getter(i, j, rows)
                    st = stat.tile([P, 3, 6], f32, tag="bnst")
                    for g in range(3):
                        nc.vector.bn_stats(out=st[0:rows, g],
                                           in_=xs[:, g * 256:(g + 1) * 256])
                    mv = stat.tile([P, 2], f32, tag="mv")
                    nc.vector.bn_aggr(out=mv[0:rows], in_=st[0:rows])
                    nc.scalar.activation(out=mv[0:rows, 1:2], in_=mv[0:rows, 1:2],
                                         func=AF.Sqrt, bias=eps_t[0:rows], scale=1.0)
                    nc.vector.reciprocal(mv[0:rows, 1:2], mv[0:rows, 1:2])
                    xc = xnp.tile([P, D], f32, tag="xnc")
                    nc.vector.tensor_scalar(out=xc[0:rows], in0=xs,
                                            scalar1=mv[0:rows, 0:1],
                                            scalar2=mv[0:rows, 1:2],
                                            op0=ALU.subtract, op1=ALU.mult)
                    nc.vector.tensor_mul(xc[0:rows], xc[0:rows], s_bc[0:rows])
                    xb = xnp.tile([P, D], bf16, tag="xnb")
                    nc.vector.tensor_add(xb[0:rows], xc[0:rows], b_bc[0:rows])
                    off = n_tok_off(i, j)
                    for k in range(DT):
                        transpose_block(xT[:, k, off:off + rows],
                                        xb[0:rows, k * P:(k + 1) * P], rows)

        def foff(i, j):
            return S * i + TOFF[j]

        # ---------------- patch embedding ----------------
        with tc.tile_pool(name="pstage", bufs=2) as pst, \
             tc.tile_pool(name="pstage1", bufs=1) as pst1, \
             tc.tile_pool(name="pdram", bufs=1, space="DRAM") as pdram:
            # im2col bounce through DRAM: patch-major, feature order (c ph pw)
            scr = pdram.tile([B * 196, D], f32)
            for b in range(B):
                for h in range(14):
                    r0 = b * 196 + h * 14
                    for c in range(3):
                        nc.sync.dma_start(
                            out=scr[r0:r0 + 14, c * 256:(c + 1) * 256].rearrange(
                                "w (ph pw) -> w ph pw", ph=16, pw=16),
                            in_=img_d[b, c, 16 * h:16 * h + 16, :].rearrange(
                                "ph (w pw) -> w ph pw", pw=16))

            # proj_w rows permuted to match feature order (c ph pw):
            # row r=(ph pw c); k-tile index k=(c, ph//8), partition p=((ph%8) pw)
            pw_b = pst1.tile([P, DT, D], bf16)
            pwsrc = projw_d[:, :].rearrange("(hi lo pw c) n -> (lo pw) c hi n",
                                            hi=2, lo=8, pw=16, c=3)
            for k in range(DT):
                wf = pst.tile([P, D], f32, tag="pwf")
                nc.sync.dma_start(out=wf[:], in_=pwsrc[:, k // 2, k % 2, :])
                nc.gpsimd.tensor_copy(out=pw_b[:, k, :], in_=wf[:])

            # pos_emb (+ proj_b on patch rows, + cls_token on CLS row)
            pe = pst1.tile([P, 2, D], f32)
            nc.sync.dma_start(out=pe[0:126, 0, :], in_=pos_d[0, 1:127, :])
            nc.sync.dma_start(out=pe[0:70, 1, :], in_=pos_d[0, 127:197, :])
            # CLS row (pos_emb[0] + cls_token) built at partition 0, DMA'd into
            # x later (compute engines can't address partition offset 70)
            clsrow = pst1.tile([1, D], f32)
            nc.sync.dma_start(out=clsrow[:], in_=pos_d[0, 0:1, :])
            clst = pst1.tile([1, D], f32)
            nc.sync.dma_start(out=clst[:], in_=cls_d[0, 0:1, :])
            nc.vector.tensor_add(clsrow[0:1, :], clsrow[0:1, :], clst[0:1, :])
            pb_bc = pst1.tile([P, D], f32)
            _bcast_row(nc, pb_bc[:], projb_d[:], P)
            nc.vector.tensor_add(pe[0:126, 0, :], pe[0:126, 0, :], pb_bc[0:126])
            nc.vector.tensor_add(pe[0:70, 1, :], pe[0:70, 1, :], pb_bc[0:70])

            # patches -> xpT (transposed, packed at 196 tokens per image)
            xpT = pst1.tile([P, DT, B * 196], bf16)
            for i in range(B):
                for j in range(2):
                    rows = PATCH_ROWS[j]
                    r0 = 196 * i + TOFF[j]
                    xf = pst.tile([P, D], f32, tag="xpf")
                    nc.sync.dma_start(out=xf[0:rows], in_=scr[r0:r0 + rows, :])
                    xb = pst.tile([P, D], bf16, tag="xpb")
                    nc.gpsimd.tensor_copy(out=xb[0:rows], in_=xf[0:rows])
                    off = 196 * i + TOFF[j]
                    for k in range(DT):
                        transpose_block(xpT[:, k, off:off + rows],
                                        xb[0:rows, k * P:(k + 1) * P], rows)

            # x = xp @ proj_w + (pos + proj_b);  CLS row = pos[0] + cls
            for i in range(B):
                for j in range(2):
                    rows = PATCH_ROWS[j]
                    off = 196 * i + TOFF[j]
                    for n in range(2):
                        ns = slice(n * 384, (n + 1) * 384)
                        ps = pmm.tile([P, 512], f32, tag="mm")
                        for k in range(DT):
                            nc.tensor.matmul(ps[0:rows, 0:384],
                                             xpT[:, k, off:off + rows],
                                             pw_b[:, k, ns],
                                             start=(k == 0), stop=(k == DT - 1))
                        nc.vector.tensor_add(x[0:rows, i, j, ns],
                                             ps[0:rows, 0:384], pe[0:rows, j, ns])
                nc.sync.dma_start(out=x[CLSROW:CLSROW + 1, i, 1, :],
                                  in_=clsrow[0:1, :])

        # ---------------- transformer layers ----------------
        lctx = ExitStack()
        act = lctx.enter_context(tc.tile_pool(name="act", bufs=1))
        xnp = lctx.enter_context(tc.tile_pool(name="xnp", bufs=2))
        attp = lctx.enter_context(tc.tile_pool(name="attp", bufs=4))
        wpool = lctx.enter_context(tc.tile_pool(name="wpool", bufs=2))
        w2pool = lctx.enter_context(tc.tile_pool(name="w2pool", bufs=1))
        lnsb = lctx.enter_context(tc.tile_pool(name="lnsb", bufs=4))
        bbc = lctx.enter_context(tc.tile_pool(name="bbc", bufs=2))

        def load_wb(src2d):
            """Stream a [768, ncol] f32 DRAM slice into a bf16 [P, DT, ncol] tile."""
            ncol = src2d.shape[1]
            wb = wpool.tile([P, DT, ncol], bf16, tag=f"wb{ncol}")
            for k in range(DT):
                wf = wpool.tile([P, ncol], f32, tag=f"wf{ncol}")
                nc.sync.dma_start(out=wf[:], in_=src2d[k * P:(k + 1) * P, :])
                nc.gpsimd.tensor_copy(out=wb[:, k, :], in_=wf[:])
            return wb

        def ln_bc(src_row):
            t = lnsb.tile([P, D], f32, tag="lnsb")
            _bcast_row(nc, t[:], src_row, P)
            return t

        for l in range(L):
            # ---- LN1 -> xnT ----
            s_bc = ln_bc(ln1s_d[l])
            b_bc = ln_bc(ln1b_d[l])
            xnT = act.tile([P, DT, TPACK], bf16, tag="xT")
            layernorm_to_T(lambda i, j, rows: x[0:rows, i, j, :],
                           s_bc, b_bc, xnT, foff)

            # ---- Q, K (transposed out), V (natural out) ----
            qT = act.tile([P, DT, TPACK], bf16, tag="qT")
            kT = act.tile([P, DT, TPACK], bf16, tag="kT")
            for (w_src, b_src, dstT) in ((wq_d, bq_d, qT), (wk_d, bk_d, kT)):
                wb = load_wb(w_src[l])
                bt = stat.tile([P, DT], f32, tag="bqk")
                nc.sync.dma_start(out=bt[:],
                                  in_=b_src[l].rearrange("(a p) -> p a", p=P))
                for m in range(DT):
                    for n in range(2):
                        ns = slice(n * 394, (n + 1) * 394)
                        ps = pmm.tile([P, 512], f32, tag="mm")
                        for k in range(DT):
                            nc.tensor.matmul(ps[:, 0:394],
                                             wb[:, k, m * P:(m + 1) * P],
                                             xnT[:, k, ns],
                                             start=(k == 0), stop=(k == DT - 1))
                        nc.vector.tensor_scalar_add(out=dstT[:, m, ns],
                                                    in0=ps[:, 0:394],
                                                    scalar1=bt[:, m:m + 1])

            v = act.tile([P, B, 2, D], bf16, tag="v")
            wb = load_wb(wv_d[l])
            bv_bc = bbc.tile([P, D], f32, tag="bbc")
            _bcast_row(nc, bv_bc[:], bv_d[l], P)
            for i in range(B):
                for j in range(2):
                    rows = NT[j]
                    off = foff(i, j)
                    for n in range(2):
                        ns = slice(n * 384, (n + 1) * 384)
                        ps = pmm.tile([P, 512], f32, tag="mm")
                        for k in range(DT):
                            nc.tensor.matmul(ps[0:rows, 0:384],
                                             xnT[:, k, off:off + rows],
                                             wb[:, k, ns],
                                             start=(k == 0), stop=(k == DT - 1))
                        nc.vector.tensor_add(v[0:rows, i, j, ns],
                                             ps[0:rows, 0:384], bv_bc[0:rows, ns])

            # ---- attention per (image, head) ----
            aoT = act.tile([P, DT, TPACK], bf16, tag="aoT")
            for i in range(B):
                for h in range(H):
                    hp = (h % 2) * DK
                    hd = h // 2
                    at = attp.tile([P, 2, S], bf16, tag="attnT")
                    for j in range(2):           # q chunks
                        qrows = NT[j]
                        qoff = foff(i, j)
                        sc = psc.tile([P, S], f32, tag="sc")
                        nc.tensor.matmul(sc[0:qrows, 0:S],
                                         qT[hp:hp + DK, hd, qoff:qoff + qrows],
                                         kT[hp:hp + DK, hd, S * i:S * i + S],
                                         start=True, stop=True)
                        mx = stat.tile([P, 1], f32, tag="mx")
                        nc.vector.tensor_reduce(out=mx[0:qrows], in_=sc[0:qrows, 0:S],
                                                axis=AX.X, op=ALU.max)
                        nc.vector.tensor_scalar_mul(out=mx[0:qrows], in0=mx[0:qrows],
                                                    scalar1=-SCALE)
                        aw = attp.tile([P, S], bf16, tag="attn")
                        den = stat.tile([P, 1], f32, tag="den")
                        nc.scalar.activation(out=aw[0:qrows, 0:S], in_=sc[0:qrows, 0:S],
                                             func=AF.Exp, bias=mx[0:qrows],
                                             scale=SCALE, accum_out=den[0:qrows])
                        nc.vector.reciprocal(den[0:qrows], den[0:qrows])
                        nc.vector.tensor_scalar_mul(out=aw[0:qrows, 0:S],
                                                    in0=aw[0:qrows, 0:S],
                                                    scalar1=den[0:qrows])
                        for j2 in range(2):      # k blocks -> attnT
                            krows = NT[j2]
                            koff = TOFF[j2]
                            t = ptr.tile([P, P], bf16, tag="tr")
                            nc.tensor.transpose(t[0:krows, 0:qrows],
                                                aw[0:qrows, koff:koff + krows],
                                                ident[0:qrows, 0:qrows])
                            nc.vector.tensor_copy(
                                out=at[0:krows, j2, TOFF[j]:TOFF[j] + qrows],
                                in_=t[0:krows, 0:qrows])
                    # attn @ V  (transposed out)
                    pa = pao.tile([DK, S], f32, tag="ao")
                    for j2 in range(2):
                        krows = NT[j2]
                        nc.tensor.matmul(pa[0:DK, 0:S],
                                         v[0:krows, i, j2, h * DK:(h + 1) * DK],
                                         at[0:krows, j2, 0:S],
                                         start=(j2 == 0), stop=(j2 == 1))
                    nc.vector.tensor_copy(out=aoT[hp:hp + DK, hd, S * i:S * i + S],
                                          in_=pa[0:DK, 0:S])

            # ---- attn out projection + residual ----
            wb = load_wb(wo_d[l])
            bo_bc = bbc.tile([P, D], f32, tag="bbc")
            _bcast_row(nc, bo_bc[:], bo_d[l], P)
            for i in range(B):
                for j in range(2):
                    rows = NT[j]
                    nc.vector.tensor_add(x[0:rows, i, j, :], x[0:rows, i, j, :],
                                         bo_bc[0:rows])
                    off = foff(i, j)
                    for n in range(2):
                        ns = slice(n * 384, (n + 1) * 384)
                        ps = pmm.tile([P, 512], f32, tag="mm")
                        for k in range(DT):
                            nc.tensor.matmul(ps[0:rows, 0:384],
                                             aoT[:, k, off:off + rows],
                                             wb[:, k, ns],
                                             start=(k == 0), stop=(k == DT - 1))
                        nc.vector.tensor_add(x[0:rows, i, j, ns],
                                             x[0:rows, i, j, ns], ps[0:rows, 0:384])

            # ---- LN2 -> xn2T ----
            s_bc = ln_bc(ln2s_d[l])
            b_bc = ln_bc(ln2b_d[l])
            xn2T = act.tile([P, DT, TPACK], bf16, tag="xT")
            layernorm_to_T(lambda i, j, rows: x[0:rows, i, j, :],
                           s_bc, b_bc, xn2T, foff)

            # ---- FFN W1 + gelu -> hT ----
            hT = act.tile([P, FT, TPACK], bf16, tag="hT")
            b1t = stat.tile([P, FT], f32, tag="b1t")
            nc.sync.dma_start(out=b1t[:], in_=b1_d[l].rearrange("(a p) -> p a", p=P))
            for g in range(4):
                wb = load_wb(w1_d[l][:, g * D:(g + 1) * D])
                for m in range(DT):
                    gm = g * DT + m
                    for n in range(2):
                        ns = slice(n * 394, (n + 1) * 394)
                        ps = pmm.tile([P, 512], f32, tag="mm")
                        for k in range(DT):
                            nc.tensor.matmul(ps[:, 0:394],
                                             wb[:, k, m * P:(m + 1) * P],
                                             xn2T[:, k, ns],
                                             start=(k == 0), stop=(k == DT - 1))
                        nc.scalar.activation(out=hT[:, gm, ns], in_=ps[:, 0:394],
                                             func=AF.Gelu, bias=b1t[:, gm:gm + 1],
                                             scale=1.0)

            # ---- FFN W2 + residual ----
            b2_bc = bbc.tile([P, D], f32, tag="bbc")
            _bcast_row(nc, b2_bc[:], b2_d[l], P)
            for i in range(B):
                for j in range(2):
                    nc.vector.tensor_add(x[0:NT[j], i, j, :], x[0:NT[j], i, j, :],
                                         b2_bc[0:NT[j]])
            for n in range(2):
                ns = slice(n * 384, (n + 1) * 384)
                w2n = w2pool.tile([P, FT, 384], bf16, tag="w2n")
                for k in range(FT):
                    wf = wpool.tile([P, 384], f32, tag="wf2")
                    nc.sync.dma_start(out=wf[:], in_=w2_d[l, k * P:(k + 1) * P, ns])
                    nc.gpsimd.tensor_copy(out=w2n[:, k, :], in_=wf[:])
                for i in range(B):
                    for j in range(2):
                        rows = NT[j]
                        off = foff(i, j)
                        ps = pmm.tile([P, 512], f32, tag="mm")
                        for k in range(FT):
                            nc.tensor.matmul(ps[0:rows, 0:384],
                                             hT[:, k, off:off + rows],
                                             w2n[:, k, :],
                                             start=(k == 0), stop=(k == FT - 1))
                        nc.vector.tensor_add(x[0:rows, i, j, ns],
                                             x[0:rows, i, j, ns], ps[0:rows, 0:384])

        lctx.close()

        # ---------------- final LN on CLS + head ----------------
        with tc.tile_pool(name="hstage", bufs=1) as hst, \
             tc.tile_pool(name="hstage2", bufs=2) as hst2:
            clsx = hst.tile([B, D], f32)
            nc.sync.dma_start(out=clsx[0:B], in_=x[CLSROW:CLSROW + 1, :, 1, :])
            st = stat.tile([P, 3, 6], f32, tag="bnst")
            for g in range(3):
                nc.vector.bn_stats(out=st[0:B, g], in_=clsx[0:B, g * 256:(g + 1) * 256])
            mv = stat.tile([P, 2], f32, tag="mv")
            nc.vector.bn_aggr(out=mv[0:B], in_=st[0:B])
            nc.scalar.activation(out=mv[0:B, 1:2], in_=mv[0:B, 1:2],
                                 func=AF.Sqrt, bias=eps_t[0:B], scale=1.0)
            nc.vector.reciprocal(mv[0:B, 1:2], mv[0:B, 1:2])
            fs_bc = hst.tile([B, D], f32)
            _bcast_row(nc, fs_bc[:], fns_d[:], B)
            fb_bc = hst.tile([B, D], f32)
            _bcast_row(nc, fb_bc[:], fnb_d[:], B)
            cn = hst.tile([B, D], f32)
            nc.vector.tensor_scalar(out=cn[0:B], in0=clsx[0:B],
                                    scalar1=mv[0:B, 0:1], scalar2=mv[0:B, 1:2],
                                    op0=ALU.subtract, op1=ALU.mult)
            nc.vector.tensor_mul(cn[0:B], cn[0:B], fs_bc[0:B])
            cnb = hst.tile([B, D], bf16)
            nc.vector.tensor_add(cnb[0:B], cn[0:B], fb_bc[0:B])
            cnT = hst.tile([P, DT, B], bf16)
            for k in range(DT):
                transpose_block(cnT[:, k, 0:B], cnb[0:B, k * P:(k + 1) * P], B)

            hwb = hst.tile([P, DT, NCLS], bf16)
            for k in range(DT):
                wf = hst2.tile([P, NCLS], f32, tag="whf")
                nc.sync.dma_start(out=wf[:], in_=hw_d[k * P:(k + 1) * P, :])
                nc.gpsimd.tensor_copy(out=hwb[:, k, :], in_=wf[:])
            hb_bc = hst.tile([B, NCLS], f32)
            _bcast_row(nc, hb_bc[:], hb_d[:], B)
            logits = hst.tile([B, NCLS], f32)
            for n in range(2):
                ns = slice(n * 500, (n + 1) * 500)
                ps = pmm.tile([P, 512], f32, tag="mm")
                for k in range(DT):
                    nc.tensor.matmul(ps[0:B, 0:500], cnT[:, k, 0:B], hwb[:, k, ns],
                                     start=(k == 0), stop=(k == DT - 1))
                nc.vector.tensor_add(logits[0:B, ns], ps[0:B, 0:500], hb_bc[0:B, ns])
            nc.sync.dma_start(out=out_d[:, :], in_=logits[0:B])

    nc.compile()
    return nc


_NC_CACHE = []


def kernel(**inputs):
    if not _NC_CACHE:
        _NC_CACHE.append(build())
    nc = _NC_CACHE[0]
    inp = {k: np.ascontiguousarray(np.asarray(v, dtype=np.float32))
           for k, v in inputs.items()}
    img = inp.pop("img")
    assert img.shape[0] == B * N_CORES
    in_maps = []
    for c in range(N_CORES):
        m = dict(inp)
        m["img"] = img[c * B:(c + 1) * B]
        in_maps.append(m)
    trace = os.environ.get("VIT_TRACE") == "1"
    res = run_bass_kernel_spmd(nc, in_maps, core_ids=list(range(N_CORES)),
                               trace=trace)
    _LAST_RESULTS.clear()
    _LAST_RESULTS.append(res)
    return np.concatenate([r["out"] for r in res.results], axis=0)


_LAST_RESULTS = []
